# revision 20
# baseline (speedup 1.0000x reference)
"""Trainium2 Bass kernel for nn_DiagonalTraining (anti-diagonal per-diag Linear).

out[b, r, c] = sum_k W[d, m, k] * xd[b, d, k] + bias[d, m],  d = r + c.

511 independent diagonals (lengths 1..256..1) sharded over 8 cores. All
streams bf16. The design minimizes HBM bytes (the kernel is DMA-bound at
~420 GB/s/core aggregate) and keeps both HWDGE rings streaming gap-free:

- Long diags n in [129,192]: SAME-LENGTH pairs (d, 510-d). Chunk-0 (k<128)
  x/W blocks are dense full-partition tiles; both chunk-1 residuals (a =
  n-128 rows each) stack into ONE dense partial-partition block [2a,
  128+N] (x residual + W residual side by side) shipped at full bandwidth
  with zero padding. The pair's two psum groups are sequential (A opens+
  closes, then B) so no interleaved-group hazard.
- Long diags n in [193,256]: standalone slots, chunk-1 as [a, 128+N]
  partial blocks, same trick.
- Shorts (n<=128): pair-packed block-diagonal bins as the baseline
  (x [128,128] stationary, W [128,128] moving, one matmul per bin).

Inputs stream first on the two HWDGE rings (greedy byte-balanced, in job
order); outputs are staged bf16 in SBUF and drain on the SWDGE ring
(gpsimd) as soon as their jobs' copies land, with the remainder on the
HWDGE rings once inputs finish. The last chunk is small so the tail is
short. Jobs execute in simulated-arrival order cycling 8 psum banks;
psum->SBUF copies alternate DVE/DVE/ACT.
"""

import sys

sys.path.insert(0, "/opt/trn_rl_repo")

import numpy as np

B, S = 128, 256
D = 2 * S - 1  # 511
NCORES = 8
NPS = 8  # psum banks cycled over jobs

TRACE = False  # test.py sets True to pull exec_time_ns from the NTFF profile
last_results = None


def _geom(d):
    r0 = max(0, d - S + 1)
    n = d + 1 if d < S else 2 * S - 1 - d
    return r0, n


def _diag_flat(d, kvals):
    r0, n = _geom(d)
    r = r0 + kvals
    return r * S + (d - r)


def _short_bins():
    sbins = []
    for kk in range(1, 64):
        sbins.append([kk - 1, 127 - kk])
        sbins.append([511 - kk, 383 + kk])
    sbins.append([63, 447])
    sbins.append([127])
    sbins.append([383])
    sbins += [[] for _ in range(136 - len(sbins))]
    return sbins


def _layout():
    """Static slot structure + schedule, shared by all cores (SPMD).

    Returns (units, jobs, CF, CP, OT, sbins).
    units: DMA units in per-ring issue order:
      {kind: 'F'|'P'|'SH', ring: 0|1, rows, cols, off (df/dp col offset)}
    jobs: execution-ordered:
      SN: {t:'SN', slot pairs[(dL,dR,n_c)] per core, N, a, fu, fo, pu, po,
           yo, w}
      ST: {t:'ST', diags[d|None], N, a, fu, fo, pu, po, yo, w}
      SH: {t:'SH', bin (global bin base idx), fu, fo, yo, w}
    """
    # ---- same-n pairs: n in [129,192] ----
    snp = [(n - 1, 511 - n, n) for n in range(192, 128, -1)]  # 64 pairs
    sn_slots = [snp[8 * u : 8 * u + 8] for u in range(8)]
    sn_N = [s[0][2] for s in sn_slots]
    # ---- standalone longs: n in [193,256] ----
    st = [255]  # n=256
    for n in range(255, 192, -1):
        st += [n - 1, 511 - n]
    st_slots = [st[8 * v : 8 * v + 8] for v in range(16)]
    st_slots[15] = st_slots[15] + [None] * (8 - len(st_slots[15]))
    st_N = [_geom(s[0])[1] for s in st_slots]
    # ---- shorts ----
    sbins = _short_bins()

    # ---- DMA units ----
    units = []

    def add_unit(kind, rows, cols):
        units.append(dict(kind=kind, rows=rows, cols=cols, ring=-1, off=0))
        return len(units) - 1

    # F units: SN slots 0-3 as per-slot units (fine-grained early PE
    # start), 4-7 merged; ST groups of 4; SH split in 2.
    fu_sn_map = {}
    for u in range(4):
        fu_sn_map[u] = add_unit("F", 128, 256 + 2 * sn_N[u])
    u47 = add_unit("F", 128, sum(256 + 2 * sn_N[u] for u in (4, 5, 6, 7)))
    for u in (4, 5, 6, 7):
        fu_sn_map[u] = u47
    fu_st = [add_unit("F", 128, sum(128 + st_N[v] for v in range(4 * g, 4 * g + 4)))
             for g in range(4)]
    fu_sh = [add_unit("F", 128, 9 * 256), add_unit("F", 128, 8 * 256)]
    # P units: per-slot for SN 0-3, merged groups after (bigger descriptors
    # amortize per-descriptor overhead). Matmul operand base partitions
    # must be in {0, 32, 64}, so the B-half of an SN P-block sits at
    # beta = 32*ceil(a/32).
    sn_beta = [32 * ((sn_N[u] - 128 + 31) // 32) for u in range(8)]
    pu_sn = [add_unit("P", sn_beta[u] + (sn_N[u] - 128), 128 + sn_N[u])
             for u in range(4)]
    p47 = add_unit("P", sn_beta[4] + (sn_N[4] - 128),
                   sum(128 + sn_N[u] for u in (4, 5, 6, 7)))
    pu_sn += [p47] * 4
    pu_st_g = [add_unit("P", st_N[4 * g] - 128,
                        sum(128 + st_N[v] for v in range(4 * g, 4 * g + 4)))
               for g in range(4)]
    pu_st = [pu_st_g[v // 4] for v in range(16)]

    # df/dp col offsets
    cf = cp = 0
    for un in units:
        if un["kind"] == "P":
            un["off"] = cp
            cp += un["cols"]
        else:
            un["off"] = cf
            cf += un["cols"]
    CF, CP = cf, cp

    # ---- ring assignment ----
    # Greedy earliest-delivery in consumption order. Ring 1 (scalar HWDGE)
    # issues its first descriptor ~2.5us after ring 0 (sync) -- model that
    # as a ~0.5MB handicap so both rings finish together and the first
    # jobs' units land on ring 0.
    order = [
        fu_sn_map[0], pu_sn[0], fu_sn_map[1], pu_sn[1],
        fu_sn_map[2], pu_sn[2], fu_sn_map[3], pu_sn[3],
        pu_sn[4], u47,
        fu_st[0], pu_st_g[0], fu_st[1], pu_st_g[1],
        fu_st[2], pu_st_g[2], fu_st[3], pu_st_g[3],
        fu_sh[0], fu_sh[1],
    ]
    rb = [0.0, 0.5e6]
    ring_units = [[], []]
    for ui in order:
        un = units[ui]
        r = 0 if rb[0] <= rb[1] else 1
        un["ring"] = r
        ring_units[r].append(ui)
        rb[r] += un["rows"] * un["cols"] * 2

    # ---- jobs ----
    jobs = []
    # per-slot F/P col offsets inside their unit
    sn_fo, off = [], {}
    for u in range(8):
        g = fu_sn_map[u]
        sn_fo.append(off.get(g, 0))
        off[g] = off.get(g, 0) + 256 + 2 * sn_N[u]
    st_fo, off = [], [0, 0, 0, 0]
    for v in range(16):
        g = v // 4
        st_fo.append(off[g])
        off[g] += 128 + st_N[v]
    sn_po, off = [], {}
    for u in range(8):
        g = pu_sn[u]
        sn_po.append(off.get(g, 0))
        off[g] = off.get(g, 0) + 128 + sn_N[u]
    st_po, off = [], [0] * 4
    for v in range(16):
        g = v // 4
        st_po.append(off[g])
        off[g] += 128 + st_N[v]

    for u in range(8):
        jobs.append(dict(t="SN", pairs=sn_slots[u], N=sn_N[u], a=sn_N[u] - 128,
                         beta=sn_beta[u], fu=fu_sn_map[u], fo=sn_fo[u],
                         pu=pu_sn[u], po=sn_po[u], w=2 * sn_N[u]))
    for v in range(16):
        jobs.append(dict(t="ST", diags=st_slots[v], N=st_N[v], a=st_N[v] - 128,
                         fu=fu_st[v // 4], fo=st_fo[v], pu=pu_st[v],
                         po=st_po[v], w=st_N[v]))
    for j in range(17):
        jobs.append(dict(t="SH", bin=j, fu=fu_sh[0] if j < 9 else fu_sh[1],
                         fo=(j if j < 9 else j - 9) * 256, w=128))

    # ---- execution order = simulated arrival order ----
    # arrival of a unit = cumulative bytes before it on its ring (equal
    # rates), with ring 1's late-start handicap
    arr = {}
    for r in (0, 1):
        c = 0.0 if r == 0 else 0.5e6
        for ui in ring_units[r]:
            un = units[ui]
            c += un["rows"] * un["cols"] * 2
            arr[ui] = c
    for k, jb in enumerate(jobs):
        a1 = arr[jb["fu"]]
        a2 = arr[jb["pu"]] if "pu" in jb else 0
        jb["arr"] = max(a1, a2)
        jb["tie"] = k
    jobs.sort(key=lambda jb: (jb["arr"], jb["tie"]))

    # yo offsets in execution order
    ot = 0
    for jb in jobs:
        jb["yo"] = ot
        ot += jb["w"]
    OT = ot
    return units, jobs, ring_units, CF, CP, OT, sbins


_UNITS, _JOBS, _RING_UNITS, CF, CP, OT, _SBINS = _layout()
N_JOBS = len(_JOBS)

# output chunks: (last_job_idx_inclusive, ring) ring: 'g'=SWDGE, 0, 1
_OUT_CHUNKS = [
    (3, "g"),
    (7, "g"),
    (11, "g"),
    (15, "g"),
    (19, "g"),
    (23, "g"),
    (27, 1),
    (31, 0),
    (35, 1),
    (38, 0),
    (N_JOBS - 1, 1),
]


def _copy_eng(k):
    """0 = DVE, 1 = ACT. DVE takes 2 of 3 (ACT also issues ring-1 DMAs)."""
    return 0 if k % 3 != 2 else 1


def _cnt(k, e):
    """#copies on engine e among jobs 0..k inclusive."""
    return sum(1 for j in range(k + 1) if _copy_eng(j) == e)


def _core_tables():
    """Per-core packing index tables (host-side).

    xgath entries: (tensor 'df'|'dp', prow, col, idx[rows] into x_flat):
      image[prow:prow+rows, col:col+128... no -- writes
      image[prow:prow+len(idx), col:col+B] = x_flat[:, idx].T
    wblk entries: (tensor, prow, col, d, m0, m1, k0, k1):
      image[prow:prow+(k1-k0), col:col+(m1-m0)] = W[d, m0:m1, k0:k1].T
    """
    cores = []
    sbins = _SBINS
    for c in range(NCORES):
        my_bins = sbins[c::NCORES]
        xgath = []
        wblk = []
        tgt = np.full(OT, -1, np.int64)
        k128 = np.arange(128)
        for jb in _JOBS:
            if jb["t"] == "SN":
                dL, dR, n_c = jb["pairs"][c]
                N, a = jb["N"], jb["a"]
                fof = _UNITS[jb["fu"]]["off"] + jb["fo"]
                pof = _UNITS[jb["pu"]]["off"] + jb["po"]
                xgath.append(("df", 0, fof, _diag_flat(dL, k128)))
                xgath.append(("df", 0, fof + 128, _diag_flat(dR, k128)))
                wblk.append(("df", 0, fof + 256, dL, 0, N, 0, 128))
                wblk.append(("df", 0, fof + 256 + N, dR, 0, N, 0, 128))
                # P block: rows [0:a) A-chunk1, rows [beta:beta+a) B-chunk1
                bta = jb["beta"]
                kk = np.minimum(128 + k128[:a], n_c - 1)
                xgath.append(("dp", 0, pof, _diag_flat(dL, kk)))
                xgath.append(("dp", bta, pof, _diag_flat(dR, kk)))
                wblk.append(("dp", 0, pof + 128, dL, 0, N, 128, 128 + a))
                wblk.append(("dp", bta, pof + 128, dR, 0, N, 128, 128 + a))
                tgt[jb["yo"]: jb["yo"] + n_c] = _diag_flat(dL, np.arange(n_c))
                tgt[jb["yo"] + N: jb["yo"] + N + n_c] = _diag_flat(dR, np.arange(n_c))
            elif jb["t"] == "ST":
                d = jb["diags"][c]
                if d is None:
                    continue
                N, a = jb["N"], jb["a"]
                fof = _UNITS[jb["fu"]]["off"] + jb["fo"]
                pof = _UNITS[jb["pu"]]["off"] + jb["po"]
                _, n_c = _geom(d)
                a_c = n_c - 128
                xgath.append(("df", 0, fof, _diag_flat(d, k128)))
                wblk.append(("df", 0, fof + 128, d, 0, N, 0, 128))
                kk = np.minimum(128 + k128[:a_c], n_c - 1)
                xgath.append(("dp", 0, pof, _diag_flat(d, kk)))
                wblk.append(("dp", 0, pof + 128, d, 0, N, 128, 128 + a_c))
                tgt[jb["yo"]: jb["yo"] + n_c] = _diag_flat(d, np.arange(n_c))
            else:  # SH
                bin_ds = my_bins[jb["bin"]]
                base = _UNITS[jb["fu"]]["off"] + jb["fo"]
                o = 0
                for d in bin_ds:
                    _, n = _geom(d)
                    i = np.arange(n)
                    xgath.append(("df", o, base, _diag_flat(d, i)))
                    wblk.append(("df", o, base + 128 + o, d, 0, n, 0, n))
                    tgt[jb["yo"] + o: jb["yo"] + o + n] = _diag_flat(d, i)
                    o += n
        cores.append(dict(xgath=xgath, wblk=wblk, tgt=tgt))
    rr, cc = np.divmod(np.arange(S * S), S)
    dd = rr + cc
    r0v = np.maximum(0, dd - S + 1)
    bidx = dd * S + (rr - r0v)
    return cores, bidx


_TABLES = None
_PROG = None


def _tables():
    global _TABLES
    if _TABLES is None:
        _TABLES = _core_tables()
    return _TABLES


def _build_program():
    import concourse.bass as bass
    import concourse.mybir as mybir

    f32 = mybir.dt.float32
    bf16 = mybir.dt.bfloat16
    nc = bass.Bass()
    df = nc.dram_tensor("df", [128, CF], bf16, kind="ExternalInput")
    dp = nc.dram_tensor("dp", [128, CP], bf16, kind="ExternalInput")
    yo = nc.dram_tensor("yo", [128, OT], bf16, kind="ExternalOutput")

    # one SBUF staging tensor per DMA unit (no WAR deps)
    BT = [
        nc.alloc_sbuf_tensor(f"bt{i}", [128, un["cols"]], bf16).ap()
        for i, un in enumerate(_UNITS)
    ]
    YO = nc.alloc_sbuf_tensor("YO", [128, OT], bf16).ap()
    PS = [nc.alloc_psum_tensor(f"ps{i}", [128, 512], f32).ap() for i in range(NPS)]

    DIN = [nc.alloc_semaphore(f"di{i}") for i in range(len(_UNITS))]
    P = nc.alloc_semaphore("P")
    CV = nc.alloc_semaphore("CV")
    CA = nc.alloc_semaphore("CA")
    DO = nc.alloc_semaphore("DO")

    def _in_dma(eng, ui):
        un = _UNITS[ui]
        src = df if un["kind"] != "P" else dp
        r = un["rows"]
        eng.dma_start(
            out=BT[ui][0:r, :],
            in_=src[0:r, un["off"]: un["off"] + un["cols"]],
        ).then_inc(DIN[ui], 16)

    def _out_dma(eng, ev):
        k, _, o0, o1 = ev
        eng.wait_ge(CV, _cnt(k, 0))
        eng.wait_ge(CA, _cnt(k, 1))
        eng.dma_start(out=yo[:, o0:o1], in_=YO[:, o0:o1]).then_inc(DO, 16)

    # resolve chunk col ranges (jobs' yo offsets are in execution order)
    out_events = []
    prev = 0
    for k, ring in _OUT_CHUNKS:
        o1 = _JOBS[k]["yo"] + _JOBS[k]["w"]
        out_events.append((k, ring, prev, o1))
        prev = o1

    def _copy(eng, sem, k):
        eng.wait_ge(P, k + 1)
        jb = _JOBS[k]
        ps = PS[k % NPS]
        dst = YO[:, jb["yo"]: jb["yo"] + jb["w"]]
        if eng is nc.vector:
            cp = eng.tensor_copy(dst, ps[:, 0: jb["w"]])
        else:
            cp = eng.copy(dst, ps[:, 0: jb["w"]])
        cp.then_inc(sem, 1)

    with nc.Block(no_gpsimd_drain=True) as block:

        @block.sync
        def _(sync):
            for ui in _RING_UNITS[0]:
                _in_dma(sync, ui)
            for ev in out_events:
                if ev[1] == 0:
                    _out_dma(sync, ev)
            sync.wait_ge(DO, 16 * len(out_events))

        @block.gpsimd
        def _(gpsimd):
            for ev in out_events:
                if ev[1] == "g":
                    _out_dma(gpsimd, ev)

        @block.scalar
        def _(scalar):
            for ui in _RING_UNITS[1]:
                _in_dma(scalar, ui)
            for k in range(N_JOBS):
                if _copy_eng(k) == 1:
                    _copy(nc.scalar, CA, k)
                for ev in out_events:
                    if ev[1] == 1 and ev[0] == k:
                        _out_dma(scalar, ev)

        @block.vector
        def _(vector):
            for k in range(N_JOBS):
                if _copy_eng(k) == 0:
                    _copy(nc.vector, CV, k)

        @block.tensor
        def _(tensor):
            waited = set()
            for k, jb in enumerate(_JOBS):
                need = [jb["fu"]] + ([jb["pu"]] if "pu" in jb else [])
                for ui in need:
                    if ui not in waited:
                        tensor.wait_ge(DIN[ui], 16)
                        waited.add(ui)
                # psum-bank recycle: batch the wait every 4 jobs (covers
                # banks for jobs k..k+3) to cut PE-sequencer wait ops
                if k >= NPS and (k - NPS) % 4 == 0:
                    prev_k = k - NPS + 3
                    tensor.wait_ge(CV, _cnt(prev_k, 0))
                    tensor.wait_ge(CA, _cnt(prev_k, 1))
                ps = PS[k % NPS]
                if jb["t"] == "SN":
                    N, a, bta = jb["N"], jb["a"], jb["beta"]
                    F = BT[jb["fu"]]
                    Pp = BT[jb["pu"]]
                    fo, po = jb["fo"], jb["po"]
                    xa = F[:, fo: fo + 128]
                    xb = F[:, fo + 128: fo + 256]
                    w0A = F[:, fo + 256: fo + 256 + N]
                    w0B = F[:, fo + 256 + N: fo + 256 + 2 * N]
                    xp = Pp[0: bta + a, po: po + 128]
                    w1 = Pp[0: bta + a, po + 128: po + 128 + N]
                    nc.tensor.matmul(ps[:, 0:N], xa, w0A, start=True, stop=False)
                    nc.tensor.matmul(
                        ps[:, 0:N], xp[0:a, :], w1[0:a, :], start=False, stop=True
                    )
                    nc.tensor.matmul(ps[:, N: 2 * N], xb, w0B, start=True, stop=False)
                    mm = nc.tensor.matmul(
                        ps[:, N: 2 * N], xp[bta: bta + a, :], w1[bta: bta + a, :],
                        start=False, stop=True,
                    )
                elif jb["t"] == "ST":
                    N, a = jb["N"], jb["a"]
                    F = BT[jb["fu"]]
                    Pp = BT[jb["pu"]]
                    fo, po = jb["fo"], jb["po"]
                    xa = F[:, fo: fo + 128]
                    w0 = F[:, fo + 128: fo + 128 + N]
                    xp = Pp[0:a, po: po + 128]
                    w1 = Pp[0:a, po + 128: po + 128 + N]
                    nc.tensor.matmul(ps[:, 0:N], xa, w0, start=True, stop=False)
                    mm = nc.tensor.matmul(ps[:, 0:N], xp, w1, start=False, stop=True)
                else:
                    F = BT[jb["fu"]]
                    fo = jb["fo"]
                    mm = nc.tensor.matmul(
                        ps[:, 0:128], F[:, fo: fo + 128], F[:, fo + 128: fo + 256],
                        start=True, stop=True,
                    )
                mm.then_inc(P, 1)

    return nc


def _get_program():
    global _PROG
    if _PROG is None:
        _PROG = _build_program()
    return _PROG


def _pack_core(t, x_flat, W, np_bf16):
    """Build df/dp images for one core."""
    imgs = {
        "df": np.zeros((128, CF), np.float32),
        "dp": np.zeros((128, CP), np.float32),
    }
    for tn, prow, col, idx in t["xgath"]:
        blk = x_flat[:, idx].T  # [len(idx) k-rows, B cols]
        imgs[tn][prow: prow + len(idx), col: col + B] = blk
    for tn, prow, col, d, m0, m1, k0, k1 in t["wblk"]:
        imgs[tn][prow: prow + (k1 - k0), col: col + (m1 - m0)] = W[
            d, m0:m1, k0:k1
        ].T
    return {k: v.astype(np_bf16) for k, v in imgs.items()}


def kernel(x, W, b):
    import ml_dtypes
    from concourse.bass_utils import run_bass_kernel_spmd

    x = np.asarray(x, np.float32)
    W = np.asarray(W, np.float32)
    b = np.asarray(b, np.float32)
    cores, bidx = _tables()
    x_flat = x.reshape(B, S * S)
    np_bf16 = ml_dtypes.bfloat16
    in_maps = [_pack_core(t, x_flat, W, np_bf16) for t in cores]
    nc = _get_program()
    res = run_bass_kernel_spmd(nc, in_maps, core_ids=list(range(NCORES)), trace=TRACE)
    global last_results
    last_results = res
    out_flat = np.zeros((B, S * S), np.float32)
    for c, t in enumerate(cores):
        yv = np.asarray(res.results[c]["yo"], np.float32).reshape(B, -1)
        fl = t["tgt"]
        vl = fl >= 0
        out_flat[:, fl[vl]] = yv[:, vl]
    out_flat += b.reshape(-1)[bidx][None, :]
    return out_flat.reshape(B, S, S)
